# revision 40
# baseline (speedup 1.0000x reference)
"""AssimilationLoss Trainium2 kernel.

Reference math (x: [B, N, D] f32):
    loss = mean_b || sum_i x[b,i,:] / max(||x[b,i,:]||, eps) ||^2 / N^2

Sharding: data-parallel over B across 8 NeuronCores (one batch element per
core).  Each core streams its [N, D] shard once from HBM (16 MiB), computes
partial_b = || sum_i x_i/||x_i|| ||^2 locally, and the host averages the 8
scalars.

Timing model (trace-derived):
  - graded window = first MEMSET (framework const setup) .. last trace event
  - the walrus codegen epilogue zeroes all ~253 semaphores behind a barrier
    (~7 us, fixed) -> total = (last engine finish) + ~7.5 us
  - wire: SWDGE saturates 330-430 GB/s (run-to-run DVFS variance); SWDGE's
    first packet lags its first trigger by ~2.4 us, so one 1-tile HWDGE DMA
    on the sync ring covers the warmup gap (more would be starved once the
    SW queue ramps; Q1 gets almost no bandwidth next to a busy Q0).

Per-core pipeline over [128, 512] row-tiles (raw Bacc, manual semaphores):
  DMA : 1-tile HWDGE (f32r) on sync first, then SWDGE chunks from gpsimd
        with f32->bf16 cast on the wire; small chunks at both ends for fine
        wait granularity, 8-tile chunks mid-stream.
  ACT : activation(Square, accum_out) -> ss[p] for half the tiles
  DVE : affine_mul_reduce -> ss[p] for the other half (both engines run
        ~660-850 ns/tile depending on DVFS; 50/50 keeps each under ~86%)
  ACT : sqrt (batched per DMA chunk)  -> norm[p]
  DVE : reciprocal                    -> inv[p] = 1/||x_p|| (bf16 / f32r)
  PE  : matmul(lhsT=inv, rhs=x_tile)  -> s[1, D] += sum_p x[p,:]/||x_p||
Epilogue: ACT square+acc of s -> scalar, DMA out from the warm sync ring.

Synchronization rules (hard-won):
  - DVE affine_mul_reduce accumulator results must be signalled by a LATER
    DVE instruction (engine_nop), never by then_inc on the amr itself, and
    never consumed by the next DVE instruction (no same-engine interlock).
  - sqrt/recip batches are emitted per chunk AFTER that chunk's squares on
    the same engine; a batch emitted after the NEXT chunk's squares sits
    behind data-gated instructions and adds a full chunk (~5 us) of inv
    latency (measured: PE starved ~4.8 us at every group boundary).
"""

import numpy as np

import concourse.bacc as bacc
import concourse.mybir as mybir
from concourse.bass_utils import run_bass_kernel_spmd


def _ensure_ntff_hook():
    """Provide antenv.axon_hooks (NTFF profiling glue) if the image lacks it."""
    try:
        from antenv.axon_hooks import get_axon_ntff_profile_hook  # noqa: F401

        return
    except ImportError:
        pass
    import contextlib
    import ctypes
    import sys
    import types

    so_path = "/opt/axon/libaxon_pjrt.so"
    mod = types.ModuleType("antenv.axon_hooks")
    _state = {"hook": None}
    mod.set_axon_ntff_profile_hook = lambda h: _state.__setitem__("hook", h)
    mod.get_axon_ntff_profile_hook = lambda: _state["hook"]
    try:
        lib = ctypes.CDLL(so_path)
        if hasattr(lib, "axon_start_nrt_profile"):
            lib.axon_start_nrt_profile.argtypes = [
                ctypes.POINTER(ctypes.c_int64),
                ctypes.c_size_t,
            ]
            lib.axon_start_nrt_profile.restype = ctypes.c_int64
            lib.axon_stop_nrt_profile.argtypes = [ctypes.c_char_p]
            lib.axon_stop_nrt_profile.restype = ctypes.c_int64

            @contextlib.contextmanager
            def _hook(output_dir, device_ids):
                import jax

                jax.devices()
                if device_ids:
                    ids = (ctypes.c_int64 * len(device_ids))(*device_ids)
                    rc = lib.axon_start_nrt_profile(ids, len(device_ids))
                else:
                    rc = lib.axon_start_nrt_profile(None, 0)
                if rc != 0:
                    raise RuntimeError(f"axon_start_nrt_profile rc={rc}")
                try:
                    yield
                finally:
                    n = lib.axon_stop_nrt_profile(str(output_dir).encode())
                    if n <= 0:
                        print(f"ntff profile: rc={n} (no files?)", file=sys.stderr)

            _state["hook"] = _hook
    except OSError:
        pass
    import antenv

    sys.modules["antenv.axon_hooks"] = mod
    antenv.axon_hooks = mod


_ensure_ntff_hook()

B, N, D = 8, 8192, 512
P = 128  # SBUF partitions

F32 = mybir.dt.float32
F32R = mybir.dt.float32r
BF16 = mybir.dt.bfloat16

# DMA plan: (n_tiles, kind).  "hs" = HWDGE from sync (f32r storage; the sync
# ring's packets start ~2.2 us before SWDGE's first packet, so a 1-tile hs
# chunk covers the SWDGE warmup gap and gives compute an early start).
# "sw" = SWDGE from gpsimd, f32 -> bf16 cast on the wire.
DMA_PLAN = (
    [(1, "hs")]
    + [(1, "sw")]
    + [(2, "sw")]
    + [(4, "sw")]
    + [(8, "sw")] * 5
    + [(4, "sw")] * 2
    + [(2, "sw")] * 2
    + [(1, "sw")] * 4
)


def _on_act(t, nt):
    """Engine for tile t's square+rowsum: strict 50/50 alternation (both
    engines cost ~660-850 ns/tile; the last 12 tiles split evenly so neither
    engine serializes the endgame).  The last tile is special-cased: its
    square is half-split across both engines."""
    if t == nt - 1:
        return False  # handled by the half-split path
    return t % 2 == 1


def _build_nc():
    nc = bacc.Bacc("TRN2", target_bir_lowering=False, debug=False)
    x_ext = nc.dram_tensor("x", [N, D], F32R, kind="ExternalInput")
    out_ext = nc.dram_tensor("out", [1, 1], F32, kind="ExternalOutput")
    _body_raw(nc, x_ext.ap(), out_ext.ap())
    nc.compile()
    return nc


def _body_raw(nc, x, out):
    assert sum(m for m, _ in DMA_PLAN) * P == N

    # per-DMA sbuf storage + tile map
    dmas = []  # (kind, ap, row0, m)
    tiles = []  # (dma_idx, i_in_dma, ap, kind)
    r0 = 0
    for di, (m, kind) in enumerate(DMA_PLAN):
        dt = BF16 if kind == "sw" else F32R
        ap = nc.alloc_sbuf_tensor(f"xt{di}", [P, m, D], dt).ap()
        dmas.append((kind, ap, r0, m))
        for i in range(m):
            tiles.append((di, i, ap, kind))
        r0 += m * P
    assert r0 == N
    NT = len(tiles)

    # sqrt/recip groups: one group per DMA chunk, so a group's sqrt/recip is
    # emitted right after that chunk's squares and never sits behind a
    # data-gated instruction of a later chunk.  The tail 1-tile chunks give
    # per-tile groups, so each tail tile's inv (and matmul) fires as soon as
    # its bytes land.
    groups = []  # (tile0, gsize, kind)
    t = 0
    for m, kind in DMA_PLAN:
        groups.append((t, m, kind))
        t += m
    assert t == NT

    ss = nc.alloc_sbuf_tensor("ss", [P, NT], F32).ap()
    nrm = nc.alloc_sbuf_tensor("nrm", [P, NT], F32).ap()
    inv_r = nc.alloc_sbuf_tensor("inv_r", [P, NT], F32R).ap()
    inv_b = nc.alloc_sbuf_tensor("inv_b", [P, NT], BF16).ap()
    ss_b = nc.alloc_sbuf_tensor("ss_b", [P, 1], F32).ap()
    sq_a = nc.alloc_sbuf_tensor("sq_a", [P, D], F32).ap()
    sq_v = nc.alloc_sbuf_tensor("sq_v", [P, D], F32).ap()
    s_sq = nc.alloc_sbuf_tensor("s_sq", [1, D], F32).ap()
    partial = nc.alloc_sbuf_tensor("partial", [1, 1], F32).ap()

    import contextlib

    _stack = contextlib.ExitStack()
    with (
        _stack,
        nc.psum_tensor([1, D], F32) as s_acc,
        nc.semaphore("amr_sem") as amr_sem,
        nc.semaphore("ssq_sem") as ssq_sem,
        nc.semaphore("norm_sem") as norm_sem,
        nc.semaphore("inv_sem") as inv_sem,
        nc.semaphore("mm_sem") as mm_sem,
        nc.semaphore("fin_sem") as fin_sem,
        nc.semaphore("out_sem") as out_sem,
        nc.Block() as block,
    ):
        dma_sems = [
            _stack.enter_context(nc.semaphore(f"dma{i}"))
            for i in range(len(DMA_PLAN))
        ]

        def dma_src(di):
            kind, ap, r0, m = dmas[di]
            return x[r0 : r0 + m * P, :].rearrange("(p n) d -> p n d", p=P)

        @block.sync
        def _(sync):
            for di, (kind, ap, r0, m) in enumerate(dmas):
                if kind == "hs":
                    sync.dma_start(out=ap, in_=dma_src(di)).then_inc(
                        dma_sems[di], 16
                    )
            # store from the warm sync HWDGE ring
            sync.wait_ge(fin_sem, 1)
            sync.dma_start(out=out, in_=partial).then_inc(out_sem, 16)
            sync.wait_ge(out_sem, 16)

        @block.gpsimd
        def _(gpsimd):
            for di, (kind, ap, r0, m) in enumerate(dmas):
                if kind == "sw":
                    gpsimd.dma_start(out=ap, in_=dma_src(di)).then_inc(
                        dma_sems[di], 16
                    )

        @block.scalar
        def _(scalar):
            # Dummy activations: pull the ACT table loads (Square/Sqrt sets)
            # into the DMA flight time instead of the first real use.
            scalar.activation(
                out=sq_a[:1, :1],
                in_=s_sq[:1, :1],
                func=mybir.ActivationFunctionType.Square,
            )
            scalar.activation(
                out=sq_a[:1, :1],
                in_=s_sq[:1, :1],
                func=mybir.ActivationFunctionType.Sqrt,
            )

            last_dma_waited = [-1]

            def tile_wait(t):
                di = tiles[t][0]
                if di > last_dma_waited[0]:
                    scalar.wait_ge(dma_sems[di], 16)
                    last_dma_waited[0] = di

            def squares(gi):
                gt0, gsize, kind = groups[gi]
                for t in range(gt0, gt0 + gsize):
                    if t == NT - 1:
                        # final tile: ACT squares the second free-dim half in
                        # parallel with DVE's first-half amr (shorter tail)
                        tile_wait(t)
                        di, i, ap, kind2 = tiles[t]
                        apf = ap.bitcast(F32) if kind2 != "sw" else ap
                        scalar.activation(
                            out=sq_a[:, : D // 2],
                            in_=apf[:, i, D // 2 :],
                            func=mybir.ActivationFunctionType.Square,
                            accum_out=ss_b,
                        ).then_inc(ssq_sem, 1)
                        continue
                    if not _on_act(t, NT):
                        continue
                    tile_wait(t)
                    di, i, ap, kind2 = tiles[t]
                    apf = ap.bitcast(F32) if kind2 != "sw" else ap
                    scalar.activation(
                        out=sq_a[:, :],
                        in_=apf[:, i, :],
                        func=mybir.ActivationFunctionType.Square,
                        accum_out=ss[:, t : t + 1],
                    ).then_inc(ssq_sem, 1)

            def sqrt(gi):
                gt0, gsize, kind = groups[gi]
                scalar.wait_ge(amr_sem, gi + 1)
                scalar.activation(
                    out=nrm[:, gt0 : gt0 + gsize],
                    in_=ss[:, gt0 : gt0 + gsize],
                    func=mybir.ActivationFunctionType.Sqrt,
                ).then_inc(norm_sem, 1)

            for gi in range(len(groups)):
                squares(gi)
                sqrt(gi)

            # epilogue: partial = sum_d s[d]^2 (single PSUM read on ACT; a
            # [1,D] f32 DMA-out costs ~0.65us more wire flight than this)
            scalar.wait_ge(mm_sem, 1)
            scalar.activation(
                out=s_sq,
                in_=s_acc.ap(),
                func=mybir.ActivationFunctionType.Square,
                accum_out=partial,
            ).then_inc(fin_sem, 1)

        @block.vector
        def _(vector):
            n_act = 0
            last_dma_waited = [-1]

            def tile_wait(t):
                di = tiles[t][0]
                if di > last_dma_waited[0]:
                    vector.wait_ge(dma_sems[di], 16)
                    last_dma_waited[0] = di

            def amrs(gi):
                nonlocal n_act
                gt0, gsize, kind = groups[gi]
                need_ssq_wait = False
                for t in range(gt0, gt0 + gsize):
                    if t == NT - 1:
                        # final tile: DVE amrs the first half; ACT's second
                        # half lands in ss_b and is added here
                        tile_wait(t)
                        di, i, ap, kind2 = tiles[t]
                        apf = ap.bitcast(F32) if kind2 != "sw" else ap
                        vector.affine_mul_reduce(
                            out=sq_v[:, : D // 2],
                            accum_out=ss[:, t : t + 1],
                            in0=apf[:, i, : D // 2],
                            in1=apf[:, i, : D // 2],
                            scale=1.0,
                            bias=0.0,
                        )
                        n_act += 1  # ACT's half-square of this tile
                        vector.wait_ge(ssq_sem, n_act)
                        vector.tensor_add(
                            ss[:, t : t + 1], ss[:, t : t + 1], ss_b
                        )
                        continue
                    if _on_act(t, NT):
                        n_act += 1
                        need_ssq_wait = True
                        continue
                    tile_wait(t)
                    di, i, ap, kind2 = tiles[t]
                    apf = ap.bitcast(F32) if kind2 != "sw" else ap
                    vector.affine_mul_reduce(
                        out=sq_v[:, :],
                        accum_out=ss[:, t : t + 1],
                        in0=apf[:, i, :],
                        in1=apf[:, i, :],
                        scale=1.0,
                        bias=0.0,
                    )
                if need_ssq_wait:
                    vector.wait_ge(ssq_sem, n_act)
                tile_wait(gt0 + gsize - 1)
                vector.engine_nop().then_inc(amr_sem, 1)

            def recip(gi):
                gt0, gsize, kind = groups[gi]
                inv = inv_b if kind == "sw" else inv_r
                vector.wait_ge(norm_sem, gi + 1)
                with nc.allow_low_precision(reason="matmul weight dtype"):
                    vector.reciprocal(
                        out=inv[:, gt0 : gt0 + gsize],
                        in_=nrm[:, gt0 : gt0 + gsize],
                    ).then_inc(inv_sem, 1)

            for gi in range(len(groups)):
                amrs(gi)
                recip(gi)

        @block.tensor
        def _(tensor):
            mm = 0
            for gi, (gt0, gsize, kind) in enumerate(groups):
                inv = inv_b if kind == "sw" else inv_r
                tensor.wait_ge(inv_sem, gi + 1)
                for t in range(gt0, gt0 + gsize):
                    di, i, ap, kind2 = tiles[t]
                    instr = tensor.matmul(
                        s_acc.ap(),
                        inv[:, t : t + 1],
                        ap[:, i, :],
                        start=(mm == 0),
                        stop=(mm == NT - 1),
                    )
                    mm += 1
            instr.then_inc(mm_sem, 1)


_NC_CACHE = {}


def _get_nc():
    if "nc" not in _NC_CACHE:
        _NC_CACHE["nc"] = _build_nc()
    return _NC_CACHE["nc"]


def kernel(x: np.ndarray, trace: bool = False):
    assert x.shape == (B, N, D), x.shape
    nc = _get_nc()
    in_maps = [{"x": np.ascontiguousarray(x[b], dtype=np.float32)} for b in range(B)]
    res = None
    for attempt in range(3):
        try:
            res = run_bass_kernel_spmd(
                nc, in_maps, core_ids=list(range(B)), trace=trace
            )
            break
        except Exception:
            # A previously crashed process can leave the accelerator in an
            # "unrecoverable" state for ~30s; it heals on its own.
            if attempt == 2:
                raise
            import time

            time.sleep(25)
    partials = [float(r["out"][0, 0]) for r in res.results]
    val = np.float32(np.sum(np.asarray(partials, dtype=np.float64)) / (N * N) / B)
    if trace:
        return val, res
    return val
